# revision 3
# baseline (speedup 1.0000x reference)
"""Trainium2 Bass kernel for nn_LoRALinear4bit — v3 (fp16 datapath).

out = x @ dequant_nf4(q_idx, absmax).T + (x @ A) @ B * 2.0
x [4,2048,4096] f32, q_idx [4096,4096] int32 (NF4 codes),
absmax [4096,64] f32, A [4096,16], B [16,4096].

Column/tensor parallel over 8 NeuronCores; per core:
  * out shard = 512 out-features; x replicated, fp16 on host (halves the
    dominant HBM stream to 67MB/core, under the 358 GB/s/core limit).
  * fp16 matmul datapath (PE full rate).  W_eff k-tiles fp16 in SBUF.
  * NF4 dequant on device: int8 codes -> u=(q-7.5)/7.5 (GPSIMD affine)
    -> DVE degree-8 LSQ polynomial of the codebook (max node err 6.9e-3,
    ~0.3% of w rms, inside the 2e-2 gate) -> absmax scale.
  * LoRA fold: W_eff = fp16(2*(A@B)^T k-tile) + poly(u)*absmax; A@B from
    tiny PE matmuls (A^T resident), emitted LOOK tiles ahead.
  * Phase B: 16 token groups of 512 tokens; NSPLIT split groups consume
    early W_eff tiles in staggered k-chunks.  PSUM cannot be read by
    GPSIMD, so partial accumulation uses the baseline trick: spill
    PSUM -> fp16 SBUF partial on ACT, re-inject via PE identity matmul
    at the next chunk.  Split finals copy out on DVE (post-dequant),
    unsplit finals on ACT.
  * x arrives as [128, KT, TOK] so each chunk is one stripe DMA (the
    HWDGE descriptor path serializes; 512 small x DMAs starve dequant).
    Output is one batched DMA per token group via a transposed DRAM AP.
  * Engine roles: DVE=Horner + split finals; ACT=spills + wadd-base
    copies + unsplit finals + out DMAs; GPSIMD=u-conv + W_eff adds;
    SYNC=x stripes + deq DMAs; PE=matmuls + injections.
"""

import numpy as np

B_, S_, IN, OUT = 4, 2048, 4096, 4096
TOK = B_ * S_            # 8192 tokens
NCORES = 8
OSH = OUT // NCORES      # 512 out-features per core
R = 16                   # LoRA rank
SCALING = 2.0            # alpha/r
QBLOCK = 64              # bnb absmax blocksize

KT = IN // 128           # 32 K tiles
TG = 512                 # token group
NG = TOK // TG           # 16 token groups
MPG = TG // 128          # 4 m-tiles per group
XSTR = 8                 # max k-tiles per x stripe DMA

DEG = 8                  # NF4 polynomial degree (LSQ on the 16 nodes)
NSPLIT = 7               # token groups consuming early k-tiles in chunks
EARLY = 5                # second-chunk step while dequant streams
STEP = 12                 # steady-state chunk size (k-tiles)
LOOK = 6                 # wadd (LoRA-fold) emission lookahead, in k-tiles

NF4 = np.array([
    -1.0, -0.6961928009986877, -0.5250730514526367, -0.39491748809814453,
    -0.28444138169288635, -0.18477343022823334, -0.09105003625154495, 0.0,
    0.07958029955625534, 0.16093020141124725, 0.24611230194568634,
    0.33791524171829224, 0.44070982933044434, 0.5626170039176941,
    0.6989699602127075, 1.0], dtype=np.float64)


def _poly_coeffs(deg=DEG):
    q = np.arange(16, dtype=np.float64)
    u = (q - 7.5) / 7.5
    V = np.vander(u, deg + 1, increasing=True)
    c, *_ = np.linalg.lstsq(V, NF4, rcond=None)
    return c


def _chunk_bounds(g):
    """Staggered k-chunk bounds for split group g: small early chunks,
    STEP-tile steady chunks."""
    b = [0, 1 + (g % 4)]
    while len(b) < 4 and b[-1] + EARLY < KT - 2:
        b.append(b[-1] + EARLY)
    while b[-1] + STEP < KT - 1:
        b.append(b[-1] + STEP)
    b.append(KT)
    return b


_CACHE = {}


def _build():
    key = "v3k"
    if key in _CACHE:
        return _CACHE[key]

    import concourse.bacc as bacc
    import concourse.tile as tile
    from concourse import mybir
    from concourse.bass import ts, ds

    f32 = mybir.dt.float32
    f16 = mybir.dt.float16
    i8 = mybir.dt.int8
    Alu = mybir.AluOpType

    c = _poly_coeffs()

    nc = bacc.Bacc("TRN2", target_bir_lowering=False, debug=False)

    xtr = nc.dram_tensor("xtr", [128, KT, TOK], f16, kind="ExternalInput").ap()
    qt = nc.dram_tensor("qt", [IN, OSH], i8, kind="ExternalInput").ap()
    scl = nc.dram_tensor("scl", [IN, OSH], f32, kind="ExternalInput").ap()
    at = nc.dram_tensor("at", [R, IN], f16, kind="ExternalInput").ap()
    bsh = nc.dram_tensor("bsh", [R, OSH], f16, kind="ExternalInput").ap()
    ident = nc.dram_tensor("ident", [128, 128], f16, kind="ExternalInput").ap()
    out = nc.dram_tensor("out", [NG, MPG, 128, OSH], f16,
                         kind="ExternalOutput").ap()

    # chunk plan: gate tile -> [(g, ci, k0, k1, nchunks)]
    gate = {j: [] for j in range(KT)}
    for g in range(NSPLIT):
        b = _chunk_bounds(g)
        for i in range(len(b) - 1):
            gate[b[i + 1] - 1].append((g, i, b[i], b[i + 1], len(b) - 1))

    with tile.TileContext(nc) as tc:
        with (
            tc.tile_pool(name="weff", bufs=1) as weff_pool,
            tc.tile_pool(name="deq", bufs=3) as deq_pool,
            tc.tile_pool(name="acc2p", bufs=8) as acc2_pool,
            tc.tile_pool(name="part", bufs=1) as part_pool,
            tc.tile_pool(name="xin", bufs=4) as x_pool,
            tc.tile_pool(name="oup", bufs=3) as o_pool,
            tc.tile_pool(name="ps", bufs=8, space="PSUM") as ps_pool,
            tc.tile_pool(name="const", bufs=1) as const_pool,
        ):
            at_sb = const_pool.tile([R, IN], f16, tag="at_sb", name="at_sb")
            nc.gpsimd.dma_start(out=at_sb[:], in_=at[:])
            b_sb = const_pool.tile([R, OSH], f16, tag="b_sb", name="b_sb")
            nc.gpsimd.dma_start(out=b_sb[:], in_=bsh[:])
            id_sb = const_pool.tile([128, 128], f16, tag="id_sb", name="id_sb")
            nc.gpsimd.dma_start(out=id_sb[:], in_=ident[:])

            weff = [weff_pool.tile([128, OSH], f16, tag=f"weff{j}",
                                   name=f"weff{j}") for j in range(KT)]
            partials = {}

            def emit_wadd(j):
                # LoRA fold base: weff_j = fp16(2*(A@B) k-tile); emitted
                # LOOK tiles ahead so the ACT copy (which trails the PE
                # backlog) lands before the dequant add needs it.
                wps = ps_pool.tile([128, OSH], f32, tag="ps", name="ps")
                nc.tensor.matmul(wps[:], at_sb[:, ts(j, 128)], b_sb[:],
                                 start=True, stop=True)
                nc.scalar.copy(weff[j][:], wps[:])

            def emit_phase_a(j):
                qtl = deq_pool.tile([128, OSH], i8, tag="qtl", name="qtl")
                sctl = deq_pool.tile([128, OSH], f32, tag="sctl", name="sctl")
                nc.sync.dma_start(out=qtl[:], in_=qt[ts(j, 128), :])
                nc.sync.dma_start(out=sctl[:], in_=scl[ts(j, 128), :])
                u = deq_pool.tile([128, OSH], f32, tag="u", name="u")
                # u = (q - 7.5) * (1/7.5) on GPSIMD (keeps ACT free for
                # PSUM spills and avoids coupling dequant to PE backlog)
                nc.gpsimd.tensor_scalar(
                    out=u[:], in0=qtl[:], scalar1=-7.5, scalar2=1.0 / 7.5,
                    op0=Alu.add, op1=Alu.mult)
                acc = deq_pool.tile([128, OSH], f32, tag="acc", name="acc")
                nc.vector.tensor_scalar_mul(acc[:], u[:], float(c[DEG]))
                for k in range(DEG - 1, 0, -1):
                    nc.vector.scalar_tensor_tensor(
                        acc[:], acc[:], float(c[k]), u[:], Alu.add, Alu.mult)
                acc2 = acc2_pool.tile([128, OSH], f16, tag="acc2",
                                      name="acc2")
                nc.vector.scalar_tensor_tensor(
                    acc2[:], acc[:], float(c[0]), sctl[:], Alu.add, Alu.mult)
                nc.gpsimd.tensor_add(weff[j][:], weff[j][:], acc2[:])

            def mm_span(g, k0, k1, start, stop, psums):
                """MMs for k in [k0,k1) with x stripe DMAs of <=XSTR tiles."""
                for s0 in range(k0, k1, XSTR):
                    s1 = min(s0 + XSTR, k1)
                    xg = x_pool.tile([128, XSTR, TG], f16, tag="xg", name="xg")
                    nc.sync.dma_start(out=xg[:, 0:s1 - s0, :],
                                      in_=xtr[:, s0:s1, ts(g, TG)])
                    for k in range(s0, s1):
                        for m in range(MPG):
                            nc.tensor.matmul(
                                psums[m][:], xg[:, k - s0, ts(m, 128)],
                                weff[k][:],
                                start=start and (k == k0),
                                stop=stop and (k == k1 - 1))

            def emit_chunk(g, ci, k0, k1, nchunks):
                psums = [ps_pool.tile([128, OSH], f32, tag="ps", name="ps")
                         for _ in range(MPG)]
                last = ci == nchunks - 1
                if ci > 0:
                    # re-inject the accumulated partial into PSUM
                    for m in range(MPG):
                        nc.tensor.matmul(psums[m][:], id_sb[:],
                                         partials[(g, m)][:],
                                         start=True, stop=False)
                mm_span(g, k0, k1, ci == 0, True, psums)
                if not last:
                    for m in range(MPG):
                        if ci == 0:
                            pt = part_pool.tile([128, OSH], f16,
                                                tag=f"part{g}_{m}",
                                                name=f"part{g}_{m}")
                            partials[(g, m)] = pt
                        nc.scalar.copy(partials[(g, m)][:], psums[m][:])
                else:
                    ot = o_pool.tile([128, MPG, OSH], f16, tag="ot",
                                     name="ot")
                    for m in range(MPG):
                        nc.vector.tensor_copy(out=ot[:, m, :],
                                              in_=psums[m][:])
                    nc.scalar.dma_start(out=out[g].transpose([1, 0, 2]),
                                        in_=ot[:])

            def emit_unsplit(g):
                psums = [ps_pool.tile([128, OSH], f32, tag="ps", name="ps")
                         for _ in range(MPG)]
                mm_span(g, 0, KT, True, True, psums)
                ot = o_pool.tile([128, MPG, OSH], f16, tag="ot", name="ot")
                for m in range(MPG):
                    nc.scalar.copy(ot[:, m, :], psums[m][:])
                nc.scalar.dma_start(out=out[g].transpose([1, 0, 2]),
                                    in_=ot[:])

            for jj in range(LOOK):
                emit_wadd(jj)
            for j in range(KT):
                if j + LOOK < KT:
                    emit_wadd(j + LOOK)
                emit_phase_a(j)
                for g, ci, k0, k1, nchunks in gate[j]:
                    emit_chunk(g, ci, k0, k1, nchunks)
            for g in range(NSPLIT, NG):
                emit_unsplit(g)

    nc.compile()
    _CACHE[key] = nc
    return nc


def _prepare_in_maps(x, q_idx, absmax, lora_A, lora_B):
    x = np.asarray(x, dtype=np.float32)
    q_idx = np.asarray(q_idx, dtype=np.int32)
    absmax = np.asarray(absmax, dtype=np.float32)
    lora_A = np.asarray(lora_A, dtype=np.float32)
    lora_B = np.asarray(lora_B, dtype=np.float32)

    # [128, KT, TOK]: xtr[r, k, t] = x[t, k*128 + r]
    xtr = np.ascontiguousarray(
        x.reshape(TOK, KT, 128).transpose(2, 1, 0).astype(np.float16))
    qt_full = q_idx.T.astype(np.int8)                          # [IN, OUT]
    at = np.ascontiguousarray(lora_A.T.astype(np.float16))     # [R, IN]
    b2 = (SCALING * lora_B).astype(np.float16)                 # [R, OUT]

    in_maps = []
    for cid in range(NCORES):
        sl = slice(cid * OSH, (cid + 1) * OSH)
        scale = np.repeat(np.ascontiguousarray(absmax[sl].T), QBLOCK, axis=0)
        in_maps.append({
            "xtr": xtr,
            "qt": np.ascontiguousarray(qt_full[:, sl]),
            "scl": np.ascontiguousarray(scale),                # [IN, OSH] f32
            "at": at,
            "bsh": np.ascontiguousarray(b2[:, sl]),
            "ident": np.eye(128, dtype=np.float16),
        })
    return in_maps


def _gather(results):
    shards = [results[cid]["out"].reshape(TOK, OSH)
              for cid in range(NCORES)]
    full = np.concatenate(shards, axis=1).astype(np.float32)   # [TOK, OUT]
    return full.reshape(B_, S_, OUT)


def kernel(x, q_idx, absmax, lora_A, lora_B):
    from concourse.bass_utils import run_bass_kernel_spmd

    nc = _build()
    in_maps = _prepare_in_maps(x, q_idx, absmax, lora_A, lora_B)
    res = run_bass_kernel_spmd(nc, in_maps, list(range(NCORES)))
    return _gather(res.results)
